# revision 57
# baseline (speedup 1.0000x reference)
"""AttentionCommModule TRN2 kernel: 8-core data-parallel single-query MHA.

Sharding: batch B=32768 split across 8 NeuronCores (4096 rows each); all
weights replicated, no collectives. Inputs are host-packed to bf16 in a
slab-chunk-major, feature-transposed layout [16, 128, bs] so each core
loads activation tiles straight into [k, b] SBUF layout with plain
contiguous DMAs (no on-chip activation transposes at all).

Engine assignment (batch-major layout, b on partitions):
  TensorE : Q/K/V projections (lhsT = transposed activation chunk
            stationary, rhs = packed weights, f32 PSUM accumulation),
            the 128x128 transposes of `weighted`, and the out-proj.
  ScalarE : all PSUM -> SBUF copies (cast to bf16), exp(), out copy.
  VectorE : QK product + big tree levels + softmax + attn*V (batched
            per 512-row macro to amortize ~0.1-0.25us/instr overheads).
  GPSIMD  : small d-reduction tree tail (SBUF-only; it cannot touch
            PSUM and is ~2-3x slower per element than DVE).
  Sync    : all DMA triggers (a trigger costs ~667ns of ScalarE time
            but is nearly free on the idle sync queue).

Software pipeline at macro (512-row) granularity, emission order per
round r:  out-proj(r-2) | loads+MMs+copies(r) | chainB(r-1) softmax
tail + attn*V | chainA(r) QK+d-reduce+exp | transposes(r-1).
Every cross-engine dependency is one full round stale so no engine's
in-order stream blocks mid-round: PE runs [outproj | MMs | transposes]
with all inputs ready, DVE's chain ops likewise. Softmax skips the
max-subtraction (scores/sqrt(d) are O(1) at this problem's weight
scale, exp cannot overflow).
"""

import numpy as np
import ml_dtypes
from contextlib import ExitStack

import concourse.bass as bass
import concourse.tile as tile
from concourse import bacc, mybir
from concourse.bass_utils import run_bass_kernel_spmd

N_CORES = 8
B_FULL = 32768
INPUT_DIM = 256
COMM = 128
NH = 4
HD = 32
N_MSGS = 7
NS = 8          # slabs = num_agents (self + 7 messages)
TILE = 128      # rows per attention tile
MACRO = 512     # rows per DMA macro-tile

BF = mybir.dt.bfloat16
F32 = mybir.dt.float32
INV_SQRT_HD = 1.0 / float(np.sqrt(HD))
PSUM_OUT_DMA = False  # PSUM is not DMA-addressable on this stack

_compiled = {}


def _build(bs: int, has_bias: bool):
    """Build + compile the per-core Bass program for a bs-row shard."""
    assert bs % MACRO == 0
    nc = bacc.Bacc(
        "TRN2",
        target_bir_lowering=False,
        debug=False,
        enable_asserts=False,
        num_devices=N_CORES,
    )
    # ktpack[c, k, b]: slab-chunk c, feature k on what becomes the SBUF
    # partition dim, batch contiguous — host pre-transposed.
    xpack = nc.dram_tensor("xpack", [2 * NS, 128, bs], BF, kind="ExternalInput").ap()
    wpack = nc.dram_tensor("wpack", [128, 2, 384], BF, kind="ExternalInput").ap()
    wod = nc.dram_tensor("wo", [128, 128], BF, kind="ExternalInput").ap()
    identd = nc.dram_tensor("ident", [128, 128], BF, kind="ExternalInput").ap()
    if has_bias:
        bkvd = nc.dram_tensor("bkv", [1, 2176], F32, kind="ExternalInput").ap()
        bod = nc.dram_tensor("bo", [1, 128], F32, kind="ExternalInput").ap()
    outd = nc.dram_tensor("out", [bs, 128], F32, kind="ExternalOutput").ap()

    with tile.TileContext(nc) as tc, ExitStack() as ctx:
        consts = ctx.enter_context(tc.tile_pool(name="consts", bufs=1))
        xtp = ctx.enter_context(tc.tile_pool(name="xtp", bufs=4))
        kvp = ctx.enter_context(tc.tile_pool(name="kvp", bufs=4))
        work = ctx.enter_context(tc.tile_pool(name="work", bufs=2))
        sm = ctx.enter_context(tc.tile_pool(name="sm", bufs=2))
        osb = ctx.enter_context(tc.tile_pool(name="osb", bufs=2))
        psum = ctx.enter_context(tc.tile_pool(name="psum", bufs=6, space="PSUM"))
        psO = ctx.enter_context(tc.tile_pool(name="psO", bufs=1, space="PSUM"))

        w_sb = consts.tile([128, 768], BF)
        nc.gpsimd.dma_start(w_sb[:].rearrange("p (c n) -> p c n", c=2), wpack[:, :, :])
        wo_sb = consts.tile([128, 128], BF)
        nc.gpsimd.dma_start(wo_sb[:], wod[:, :])
        ident_sb = consts.tile([128, 128], BF)
        nc.gpsimd.dma_start(ident_sb[:], identd[:, :])
        if has_bias:
            bkv_sb = consts.tile([1, 2176], F32)
            nc.gpsimd.dma_start(bkv_sb[:], bkvd[:, :])
            bo_sb = consts.tile([1, 128], F32)
            nc.gpsimd.dma_start(bo_sb[:], bod[:, :])

        def emit_load(m, row0, rows):
            # ---- input load: coalesced DMAs on the idle sync engine ----
            # (a DMA trigger costs ~667ns of ScalarE time vs ~600ns on the
            # otherwise-idle sync queue)
            xt = xtp.tile([128, 16 * MACRO], BF, name=f"xt{m}", tag="xt")
            xtv = xt[:, 0:16 * rows]
            H = rows // 2
            for half in (0, 1):
                nc.sync.dma_start(
                    xtv.rearrange("p (c n) -> p c n", c=16)
                    [:, :, half * H:(half + 1) * H],
                    xpack[:, :, row0 + half * H:
                          row0 + (half + 1) * H].transpose([1, 0, 2]),
                )
            return xt

        KV = 2176  # per-tile [Q | K0 V0 | ... | K7 V7] span in kv4

        def emit_mm_tile(m, j, xt, kv4, rows):
            # PSUM layout: [Q | K0 V0 | K1 V1 ... K7 V7]
            psA = psum.tile([128, 384], F32, tag="ps", name=f"psA{m}_{j}")
            for ch in (0, 1):
                nc.tensor.matmul(
                    psA[:, :],
                    lhsT=xt[:, ch * rows + j * TILE: ch * rows + j * TILE + 128],
                    rhs=w_sb[:, ch * 384:(ch + 1) * 384],
                    start=(ch == 0),
                    stop=(ch == 1),
                )
            kvtiles = [psA]
            for pi, pair in enumerate(((1, 2), (3, 4), (5, 6), (7,))):
                width = 256 * len(pair)
                ps = psum.tile([128, width], F32, tag="ps", name=f"ps{pi}_{m}_{j}")
                for si, s in enumerate(pair):
                    for ch in (0, 1):
                        nc.tensor.matmul(
                            ps[:, si * 256:(si + 1) * 256],
                            lhsT=xt[:, (2 * s + ch) * rows + j * TILE:
                                    (2 * s + ch) * rows + j * TILE + 128],
                            rhs=w_sb[:, ch * 384 + 128:(ch + 1) * 384],
                            start=(ch == 0),
                            stop=(ch == 1),
                        )
                kvtiles.append(ps)

            # PSUM -> SBUF copies: ScalarE (4 big) + DVE (last small one)
            off = j * KV
            for pi, ps in enumerate(kvtiles):
                w = ps.shape[1]
                if pi == len(kvtiles) - 1 and j % 2 == 0:
                    nc.vector.tensor_copy(kv4[:, off:off + w], ps[:, :])
                else:
                    nc.scalar.copy(kv4[:, off:off + w], ps[:, :])
                off += w
            if has_bias:
                nc.vector.tensor_add(
                    kv4[:, j * KV:(j + 1) * KV],
                    kv4[:, j * KV:(j + 1) * KV],
                    bkv_sb[:, :].partition_broadcast(128),
                )

        def emit_chainA(m, st):
            """p1 = Q*K, d-reduction, exp — consumed one round later.
            Emitted in two tile-halves so the first half's DVE work starts
            as soon as the first tiles' copies land, instead of idling
            until ScalarE finishes the whole macro's copies."""
            nt = st["nt"]
            kv4 = st["kv4"]
            p1 = work.tile([128, 4096], BF, tag="p1", name=f"p1_{m}")
            scores4 = sm.tile([128, 128], F32, tag="scores4", name=f"sc4_{m}")
            e4 = sm.tile([128, 128], BF, tag="e4", name=f"e4_{m}")
            halves = [(0, nt)] if nt == 1 else [(0, nt // 2), (nt // 2, nt)]
            for hi, (t0, t1) in enumerate(halves):
                th = t1 - t0
                Gh = th * NS * NH
                g0 = t0 * NS * NH
                kv4r = kv4[:, t0 * KV:t1 * KV].rearrange(
                    "p (t x) -> p t x", t=th)
                qb = (
                    kv4r[:, :, 0:128]
                    .unsqueeze(2)
                    .broadcast_to([128, th, NS, 128])
                )
                kk = kv4r[:, :, 128:KV].rearrange(
                    "p t (s kv c) -> p t s kv c", s=NS, kv=2
                )[:, :, :, 0, :]
                nc.vector.tensor_mul(
                    p1[:, t0 * 1024:t1 * 1024].rearrange(
                        "p (t s c) -> p t s c", t=th, s=NS),
                    qb, kk,
                )
                cur, coff, cd = p1, t0 * 1024, HD
                for r in range(3):
                    nxt = work.tile(
                        [128, 2 * NS * NH * cd // 2], BF, tag=f"t{r}{hi}",
                        name=f"t{r}{hi}_{m}",
                    )
                    v = cur[:, coff:coff + Gh * cd].rearrange(
                        "p (g e d) -> p g e d", g=Gh, e=2)
                    eng = nc.gpsimd if r == 1 else nc.vector
                    eng.tensor_add(
                        nxt[:, 0:Gh * cd // 2].rearrange(
                            "p (g d) -> p g d", g=Gh),
                        v[:, :, 0, :],
                        v[:, :, 1, :],
                    )
                    cur, coff, cd = nxt, 0, cd // 2
                nc.vector.reduce_sum(
                    scores4[:, g0:g0 + Gh].rearrange(
                        "p (g o) -> p g o", g=Gh),
                    cur[:, 0:Gh * 4].rearrange("p (g d) -> p g d", g=Gh),
                    axis=mybir.AxisListType.X,
                )
                nc.scalar.activation(
                    e4[:, g0:g0 + Gh], scores4[:, g0:g0 + Gh],
                    mybir.ActivationFunctionType.Exp,
                    scale=INV_SQRT_HD,
                )
            st["e4"] = e4

        def emit_chainB(m, st):
            """softmax normalization + attn*V + s-reduction -> wtd4."""
            nt = st["nt"]
            G = nt * NS * NH
            kv4r = st["kv4"][:, 0:nt * KV].rearrange("p (t x) -> p t x", t=nt)
            e4 = st["e4"]
            s4 = sm.tile([128, 16], F32, tag="s4", name=f"s4_{m}")
            e_tsh = e4[:, 0:G].rearrange("p (t s h) -> p t s h", t=nt, s=NS)
            nc.vector.reduce_sum(
                s4[:, 0:nt * NH].rearrange("p (t h) -> p t h", t=nt),
                e_tsh.transpose([0, 1, 3, 2]),
                axis=mybir.AxisListType.X)
            r4 = sm.tile([128, 16], F32, tag="r4", name=f"r4_{m}")
            nc.vector.reciprocal(r4[:, 0:nt * NH], s4[:, 0:nt * NH])
            a4 = sm.tile([128, 128], BF, tag="a4", name=f"a4_{m}")
            r4b = (
                r4[:, 0:nt * NH]
                .rearrange("p (t h) -> p t h", t=nt)
                .unsqueeze(2)
                .broadcast_to([128, nt, NS, NH])
            )
            nc.vector.tensor_mul(
                a4[:, 0:G].rearrange("p (t s h) -> p t s h", t=nt, s=NS),
                e_tsh, r4b
            )
            # p2 = attn * V, laid out (s, t, d, h) so the s-reduction tree is
            # contiguous halves; V columns are d-major so the attn broadcast
            # is stride-1 innermost (DVE 2x). One mul per t (3-dim AP limit).
            p2 = work.tile([128, 4096], BF, tag="p2", name=f"p2_{m}")
            p2v = p2[:, 0:nt * NS * 128].rearrange("p (s t c) -> p s t c",
                                                   s=NS, t=nt)
            a4r = a4[:, 0:G].rearrange("p (t s h) -> p t s h", t=nt, s=NS)
            vvr = kv4r[:, :, 128:KV].rearrange(
                "p t (s kv d h) -> p t s kv d h", s=NS, kv=2, d=HD
            )
            for t in range(nt):
                ab_t = (
                    a4r[:, t, :, :]
                    .unsqueeze(2)
                    .broadcast_to([128, NS, HD, NH])
                )
                nc.vector.tensor_mul(
                    p2v[:, :, t, :].rearrange("p s (d h) -> p s d h", d=HD),
                    ab_t, vvr[:, t, :, 1, :, :],
                )
            # s-reduction tree: big level on DVE (2x bf16); the small
            # contiguous 1-D tail levels on the idle GPSIMD
            cur, cn = p2, NS
            for r in range(3):
                nxt = work.tile(
                    [128, cn * 256], BF, tag=f"v{r}", name=f"v{r}_{m}"
                )
                W = cn * nt * 64
                nc.vector.tensor_add(
                    nxt[:, 0:W], cur[:, 0:W], cur[:, W:2 * W]
                )
                cur, cn = nxt, cn // 2
            st["wtd4"] = cur  # [128, nt*128] bf16, (t, c), c=(d,h) d-major

        def emit_transposes(m, st):
            # PE reaches these after the round's MMs; wtd4 (chainB, emitted
            # earlier this round on DVE) is ready by then. The wtdT4 SBUF
            # copy is DVE's last op of the round; its consumer (out-proj) is
            # scheduled at the TOP of the next PE round.
            nt = st["nt"]
            wtd4 = st["wtd4"]
            ptp4 = psO.tile([128, 512], BF, tag="ptp4", name=f"ptp4_{m}")
            for t in range(nt):
                nc.tensor.transpose(ptp4[:, t * 128:(t + 1) * 128],
                                    wtd4[:, t * 128:(t + 1) * 128],
                                    ident_sb[:])
            wtdT4 = work.tile([128, 512], BF, tag="wtdT4", name=f"wtdT4_{m}")
            nc.vector.tensor_copy(wtdT4[:, 0:nt * 128], ptp4[:, 0:nt * 128])
            st["wtdT4"] = wtdT4

        def emit_outproj(m, st):
            nt, row0 = st["nt"], st["row0"]
            wtdT4 = st["wtdT4"]
            po4 = psO.tile([128, 512], F32, tag="po4", name=f"po4_{m}")
            for t in range(nt):
                nc.tensor.matmul(po4[:, t * 128:(t + 1) * 128],
                                 lhsT=wtdT4[:, t * 128:(t + 1) * 128],
                                 rhs=wo_sb[:], start=True, stop=True)
            out_sb = osb.tile([128, 4 * TILE], F32, tag="out_sb",
                              name=f"osb{m}")
            nc.scalar.copy(out_sb[:, 0:nt * 128], po4[:, 0:nt * 128])
            if has_bias:
                nc.vector.tensor_add(
                    out_sb[:, 0:nt * 128].rearrange("p (t j) -> p t j", t=nt),
                    out_sb[:, 0:nt * 128].rearrange("p (t j) -> p t j", t=nt),
                    bo_sb[:, :].partition_broadcast(128).unsqueeze(1)
                    .broadcast_to([128, nt, 128]),
                )
            nc.sync.dma_start(
                outd[row0:row0 + nt * TILE, :].rearrange(
                    "(t p) j -> p t j", t=nt
                ),
                out_sb[:, 0:nt * 128].rearrange("p (t j) -> p t j", t=nt),
            )

        # Software pipeline at macro granularity. Emission order per round r:
        #   1. out-proj(r-2) + output copy/DMA   (inputs one round stale)
        #   2. loads + projection MMs + PSUM->SBUF copies (r)
        #   3. chainB(r-1): softmax tail + attn*V  (e4 one round stale)
        #   4. chainA(r): QK + d-reduce + exp     (kv4 copies same round)
        #   5. transposes(r-1)                    (wtd4 from step 3)
        # PE's stream [outproj | MMs | transposes] never waits on same-round
        # work; DVE's only same-round dependency (p1 on the copies) is
        # buffered by chainB work in front of it.
        # Macro schedule: short first/last macros (2 tiles) trim the
        # pipeline fill and drain; 512-row macros in the middle.
        if bs >= 2048 and (bs - 1024) % MACRO == 0:
            sizes = [256] + [512] * ((bs - 1024) // 512) + [256] * 3
        else:
            sizes = [512] * (bs // 512)
        macros = []
        row0 = 0
        for rows in sizes:
            macros.append((row0, rows))
            row0 += rows
        n_macro = len(macros)

        state = {}
        xts = {}

        def ensure_load(m):
            # Prefetch: issue macro m's load the round BEFORE its matmuls,
            # and ahead of the out-DMA trigger in the sync engine's in-order
            # stream (that trigger waits on same-round ScalarE work, which
            # used to delay the next load by most of a round).
            if m < n_macro and m not in xts:
                row0, rows = macros[m]
                xts[m] = emit_load(m, row0, rows)

        for r in range(n_macro + 2):
            ensure_load(r)
            ensure_load(r + 1)
            if 0 <= r - 2 < n_macro:
                emit_outproj(r - 2, state[r - 2])
                del state[r - 2]
            if r < n_macro:
                row0, rows = macros[r]
                nt = rows // TILE
                st = state[r] = {
                    "xt": xts.pop(r),
                    "kv4": kvp.tile([128, 4 * KV], BF, tag="kv4",
                                    name=f"kv4_{r}"),
                    "nt": nt,
                    "row0": row0,
                }
                for j in range(nt):
                    emit_mm_tile(r, j, st["xt"], st["kv4"], rows)
            if 0 <= r - 1 < n_macro:
                emit_chainB(r - 1, state[r - 1])
            if r < n_macro:
                emit_chainA(r, state[r])
            if 0 <= r - 1 < n_macro:
                emit_transposes(r - 1, state[r - 1])

    nc.compile()
    return nc


def _get_compiled(bs: int, has_bias: bool):
    key = (bs, has_bias)
    if key not in _compiled:
        _compiled[key] = _build(bs, has_bias)
    return _compiled[key]


def _pack_inputs(agent_obs, messages, Wq, bq, Wk, bk, Wv, bv, Wo, bo):
    """Host-side packing (per full batch): returns dict of device arrays."""
    bf16 = ml_dtypes.bfloat16
    b = agent_obs.shape[0]
    allm = np.concatenate([agent_obs[:, None, :], messages], axis=1)  # [b, 8, 256]
    # slab-chunk-major, feature-transposed: xpack[2s+ch, k, b]
    xpack = np.ascontiguousarray(
        allm.reshape(b, NS, 2, 128).transpose(1, 2, 3, 0).reshape(16, 128, b)
    ).astype(bf16)

    # V (and Wo rows) in d-major column order c' = d*NH + h so the DVE
    # attn broadcast is stride-1 innermost.
    perm = (np.arange(128).reshape(NH, HD).T).reshape(-1)  # c' -> h*HD+d
    WvTp = Wv.T[:, perm]
    wcat = np.concatenate([Wq.T, Wk.T, WvTp], axis=1)  # [256, 384]
    wpack = np.ascontiguousarray(
        wcat.reshape(2, 128, 384).transpose(1, 0, 2)
    ).astype(bf16)  # [128, 2, 384]
    wo = np.ascontiguousarray(Wo.T[perm, :]).astype(bf16)  # [128, 128]

    has_bias = bool(
        np.any(bq != 0) or np.any(bk != 0) or np.any(bv != 0) or np.any(bo != 0)
    )
    extra = {"ident": np.eye(128, dtype=bf16)}
    if has_bias:
        # PSUM layout [Q | K0 V0 | ... | K7 V7]
        bkv = np.zeros((1, 2176), np.float32)
        bkv[0, 0:128] = bq
        for s in range(NS):
            bkv[0, 128 + s * 256:128 + s * 256 + 128] = bk
            bkv[0, 256 + s * 256:256 + s * 256 + 128] = bv[perm]
        extra["bkv"] = bkv
        extra["bo"] = bo.reshape(1, 128).astype(np.float32)
    return xpack, wpack, wo, extra, has_bias


def kernel(agent_obs, messages, Wq, bq, Wk, bk, Wv, bv, Wo, bo):
    b = agent_obs.shape[0]
    assert b % N_CORES == 0
    bs = b // N_CORES

    xpack, wpack, wo, extra, has_bias = _pack_inputs(
        np.asarray(agent_obs, np.float32), np.asarray(messages, np.float32),
        np.asarray(Wq, np.float32), np.asarray(bq, np.float32),
        np.asarray(Wk, np.float32), np.asarray(bk, np.float32),
        np.asarray(Wv, np.float32), np.asarray(bv, np.float32),
        np.asarray(Wo, np.float32), np.asarray(bo, np.float32),
    )
    nc = _get_compiled(bs, has_bias)

    in_maps = []
    for c in range(N_CORES):
        m = {
            "xpack": np.ascontiguousarray(xpack[:, :, c * bs:(c + 1) * bs]),
            "wpack": wpack,
            "wo": wo,
        }
        m.update(extra)
        in_maps.append(m)

    res = run_bass_kernel_spmd(nc, in_maps, core_ids=list(range(N_CORES)))
    out = np.concatenate([r["out"] for r in res.results], axis=0)
    return out.astype(np.float32)



# revision 58
# speedup vs baseline: 1.0008x; 1.0008x over previous
"""AttentionCommModule TRN2 kernel: 8-core data-parallel single-query MHA.

Sharding: batch B=32768 split across 8 NeuronCores (4096 rows each); all
weights replicated, no collectives. Inputs are host-packed to bf16 in a
slab-chunk-major, feature-transposed layout [16, 128, bs] so each core
loads activation tiles straight into [k, b] SBUF layout with plain
contiguous DMAs (no on-chip activation transposes at all).

Engine assignment (batch-major layout, b on partitions):
  TensorE : Q/K/V projections (lhsT = transposed activation chunk
            stationary, rhs = packed weights, f32 PSUM accumulation),
            the 128x128 transposes of `weighted`, and the out-proj.
  ScalarE : all PSUM -> SBUF copies (cast to bf16), exp(), out copy.
  VectorE : QK product + big tree levels + softmax + attn*V (batched
            per 512-row macro to amortize ~0.1-0.25us/instr overheads).
  GPSIMD  : small d-reduction tree tail (SBUF-only; it cannot touch
            PSUM and is ~2-3x slower per element than DVE).
  Sync    : all DMA triggers (a trigger costs ~667ns of ScalarE time
            but is nearly free on the idle sync queue).

Software pipeline at macro (512-row) granularity, emission order per
round r:  out-proj(r-2) | loads+MMs+copies(r) | chainB(r-1) softmax
tail + attn*V | chainA(r) QK+d-reduce+exp | transposes(r-1).
Every cross-engine dependency is one full round stale so no engine's
in-order stream blocks mid-round: PE runs [outproj | MMs | transposes]
with all inputs ready, DVE's chain ops likewise. Softmax skips the
max-subtraction (scores/sqrt(d) are O(1) at this problem's weight
scale, exp cannot overflow).
"""

import numpy as np
import ml_dtypes
from contextlib import ExitStack

import concourse.bass as bass
import concourse.tile as tile
from concourse import bacc, mybir
from concourse.bass_utils import run_bass_kernel_spmd

N_CORES = 8
B_FULL = 32768
INPUT_DIM = 256
COMM = 128
NH = 4
HD = 32
N_MSGS = 7
NS = 8          # slabs = num_agents (self + 7 messages)
TILE = 128      # rows per attention tile
MACRO = 512     # rows per DMA macro-tile

BF = mybir.dt.bfloat16
F32 = mybir.dt.float32
INV_SQRT_HD = 1.0 / float(np.sqrt(HD))
PSUM_OUT_DMA = False  # PSUM is not DMA-addressable on this stack

_compiled = {}


def _build(bs: int, has_bias: bool):
    """Build + compile the per-core Bass program for a bs-row shard."""
    assert bs % MACRO == 0
    nc = bacc.Bacc(
        "TRN2",
        target_bir_lowering=False,
        debug=False,
        enable_asserts=False,
        num_devices=N_CORES,
    )
    # ktpack[c, k, b]: slab-chunk c, feature k on what becomes the SBUF
    # partition dim, batch contiguous — host pre-transposed.
    xpack = nc.dram_tensor("xpack", [2 * NS, 128, bs], BF, kind="ExternalInput").ap()
    wpack = nc.dram_tensor("wpack", [128, 2, 384], BF, kind="ExternalInput").ap()
    wod = nc.dram_tensor("wo", [128, 128], BF, kind="ExternalInput").ap()
    identd = nc.dram_tensor("ident", [128, 128], BF, kind="ExternalInput").ap()
    if has_bias:
        bkvd = nc.dram_tensor("bkv", [1, 2176], F32, kind="ExternalInput").ap()
        bod = nc.dram_tensor("bo", [1, 128], F32, kind="ExternalInput").ap()
    outd = nc.dram_tensor("out", [bs, 128], F32, kind="ExternalOutput").ap()

    with tile.TileContext(nc) as tc, ExitStack() as ctx:
        consts = ctx.enter_context(tc.tile_pool(name="consts", bufs=1))
        xtp = ctx.enter_context(tc.tile_pool(name="xtp", bufs=4))
        kvp = ctx.enter_context(tc.tile_pool(name="kvp", bufs=4))
        work = ctx.enter_context(tc.tile_pool(name="work", bufs=2))
        sm = ctx.enter_context(tc.tile_pool(name="sm", bufs=2))
        osb = ctx.enter_context(tc.tile_pool(name="osb", bufs=2))
        psum = ctx.enter_context(tc.tile_pool(name="psum", bufs=6, space="PSUM"))
        psO = ctx.enter_context(tc.tile_pool(name="psO", bufs=1, space="PSUM"))

        w_sb = consts.tile([128, 768], BF)
        nc.gpsimd.dma_start(w_sb[:].rearrange("p (c n) -> p c n", c=2), wpack[:, :, :])
        wo_sb = consts.tile([128, 128], BF)
        nc.gpsimd.dma_start(wo_sb[:], wod[:, :])
        ident_sb = consts.tile([128, 128], BF)
        nc.gpsimd.dma_start(ident_sb[:], identd[:, :])
        if has_bias:
            bkv_sb = consts.tile([1, 2176], F32)
            nc.gpsimd.dma_start(bkv_sb[:], bkvd[:, :])
            bo_sb = consts.tile([1, 128], F32)
            nc.gpsimd.dma_start(bo_sb[:], bod[:, :])

        def emit_load(m, row0, rows):
            # ---- input load: coalesced DMAs on the idle sync engine ----
            # (a DMA trigger costs ~667ns of ScalarE time vs ~600ns on the
            # otherwise-idle sync queue)
            xt = xtp.tile([128, 16 * MACRO], BF, name=f"xt{m}", tag="xt")
            xtv = xt[:, 0:16 * rows]
            H = rows // 2
            for half in (0, 1):
                nc.sync.dma_start(
                    xtv.rearrange("p (c n) -> p c n", c=16)
                    [:, :, half * H:(half + 1) * H],
                    xpack[:, :, row0 + half * H:
                          row0 + (half + 1) * H].transpose([1, 0, 2]),
                )
            return xt

        KV = 2176  # per-tile [Q | K0 V0 | ... | K7 V7] span in kv4

        def emit_mm_tile(m, j, xt, kv4, rows):
            # PSUM layout: [Q | K0 V0 | K1 V1 ... K7 V7]
            psA = psum.tile([128, 384], F32, tag="ps", name=f"psA{m}_{j}")
            for ch in (0, 1):
                nc.tensor.matmul(
                    psA[:, :],
                    lhsT=xt[:, ch * rows + j * TILE: ch * rows + j * TILE + 128],
                    rhs=w_sb[:, ch * 384:(ch + 1) * 384],
                    start=(ch == 0),
                    stop=(ch == 1),
                )
            kvtiles = [psA]
            for pi, pair in enumerate(((1, 2), (3, 4), (5, 6), (7,))):
                width = 256 * len(pair)
                ps = psum.tile([128, width], F32, tag="ps", name=f"ps{pi}_{m}_{j}")
                for si, s in enumerate(pair):
                    for ch in (0, 1):
                        nc.tensor.matmul(
                            ps[:, si * 256:(si + 1) * 256],
                            lhsT=xt[:, (2 * s + ch) * rows + j * TILE:
                                    (2 * s + ch) * rows + j * TILE + 128],
                            rhs=w_sb[:, ch * 384 + 128:(ch + 1) * 384],
                            start=(ch == 0),
                            stop=(ch == 1),
                        )
                kvtiles.append(ps)

            # PSUM -> SBUF copies: ScalarE (4 big) + DVE (last small one)
            off = j * KV
            for pi, ps in enumerate(kvtiles):
                w = ps.shape[1]
                if pi == len(kvtiles) - 1 and j % 2 == 0:
                    nc.vector.tensor_copy(kv4[:, off:off + w], ps[:, :])
                else:
                    nc.scalar.copy(kv4[:, off:off + w], ps[:, :])
                off += w
            if has_bias:
                nc.vector.tensor_add(
                    kv4[:, j * KV:(j + 1) * KV],
                    kv4[:, j * KV:(j + 1) * KV],
                    bkv_sb[:, :].partition_broadcast(128),
                )

        def emit_chainA(m, st):
            """p1 = Q*K, d-reduction, exp — consumed one round later.
            Emitted in two tile-halves so the first half's DVE work starts
            as soon as the first tiles' copies land, instead of idling
            until ScalarE finishes the whole macro's copies."""
            nt = st["nt"]
            kv4 = st["kv4"]
            p1 = work.tile([128, 4096], BF, tag="p1", name=f"p1_{m}")
            scores4 = sm.tile([128, 128], F32, tag="scores4", name=f"sc4_{m}")
            e4 = sm.tile([128, 128], BF, tag="e4", name=f"e4_{m}")
            halves = [(0, nt)] if nt == 1 else [(0, nt // 2), (nt // 2, nt)]
            # Phase 1 per half: p1 + tree levels 0 (DVE) and 1 (GPSIMD).
            # Phase 2 per half: tree level 2 + reduce + exp. Interleaving
            # the halves this way means DVE never waits on the GPSIMD
            # handoff: half-b's p1/dL0 fill the gap while gp runs dL1a.
            hstate = []
            for hi, (t0, t1) in enumerate(halves):
                th = t1 - t0
                Gh = th * NS * NH
                kv4r = kv4[:, t0 * KV:t1 * KV].rearrange(
                    "p (t x) -> p t x", t=th)
                qb = (
                    kv4r[:, :, 0:128]
                    .unsqueeze(2)
                    .broadcast_to([128, th, NS, 128])
                )
                kk = kv4r[:, :, 128:KV].rearrange(
                    "p t (s kv c) -> p t s kv c", s=NS, kv=2
                )[:, :, :, 0, :]
                nc.vector.tensor_mul(
                    p1[:, t0 * 1024:t1 * 1024].rearrange(
                        "p (t s c) -> p t s c", t=th, s=NS),
                    qb, kk,
                )
                cur, coff, cd = p1, t0 * 1024, HD
                for r in range(2):
                    nxt = work.tile(
                        [128, 2 * NS * NH * cd // 2], BF, tag=f"t{r}{hi}",
                        name=f"t{r}{hi}_{m}",
                    )
                    v = cur[:, coff:coff + Gh * cd].rearrange(
                        "p (g e d) -> p g e d", g=Gh, e=2)
                    eng = nc.gpsimd if r == 1 else nc.vector
                    eng.tensor_add(
                        nxt[:, 0:Gh * cd // 2].rearrange(
                            "p (g d) -> p g d", g=Gh),
                        v[:, :, 0, :],
                        v[:, :, 1, :],
                    )
                    cur, coff, cd = nxt, 0, cd // 2
                hstate.append((cur, cd))
            for hi, (t0, t1) in enumerate(halves):
                th = t1 - t0
                Gh = th * NS * NH
                g0 = t0 * NS * NH
                cur, cd = hstate[hi]
                nxt = work.tile(
                    [128, 2 * NS * NH * cd // 2], BF, tag=f"t2{hi}",
                    name=f"t2{hi}_{m}",
                )
                v = cur[:, 0:Gh * cd].rearrange(
                    "p (g e d) -> p g e d", g=Gh, e=2)
                nc.vector.tensor_add(
                    nxt[:, 0:Gh * cd // 2].rearrange(
                        "p (g d) -> p g d", g=Gh),
                    v[:, :, 0, :],
                    v[:, :, 1, :],
                )
                nc.vector.reduce_sum(
                    scores4[:, g0:g0 + Gh].rearrange(
                        "p (g o) -> p g o", g=Gh),
                    nxt[:, 0:Gh * 4].rearrange("p (g d) -> p g d", g=Gh),
                    axis=mybir.AxisListType.X,
                )
                nc.scalar.activation(
                    e4[:, g0:g0 + Gh], scores4[:, g0:g0 + Gh],
                    mybir.ActivationFunctionType.Exp,
                    scale=INV_SQRT_HD,
                )
            st["e4"] = e4

        def emit_chainB(m, st):
            """softmax normalization + attn*V + s-reduction -> wtd4."""
            nt = st["nt"]
            G = nt * NS * NH
            kv4r = st["kv4"][:, 0:nt * KV].rearrange("p (t x) -> p t x", t=nt)
            e4 = st["e4"]
            s4 = sm.tile([128, 16], F32, tag="s4", name=f"s4_{m}")
            e_tsh = e4[:, 0:G].rearrange("p (t s h) -> p t s h", t=nt, s=NS)
            nc.vector.reduce_sum(
                s4[:, 0:nt * NH].rearrange("p (t h) -> p t h", t=nt),
                e_tsh.transpose([0, 1, 3, 2]),
                axis=mybir.AxisListType.X)
            r4 = sm.tile([128, 16], F32, tag="r4", name=f"r4_{m}")
            nc.vector.reciprocal(r4[:, 0:nt * NH], s4[:, 0:nt * NH])
            a4 = sm.tile([128, 128], BF, tag="a4", name=f"a4_{m}")
            r4b = (
                r4[:, 0:nt * NH]
                .rearrange("p (t h) -> p t h", t=nt)
                .unsqueeze(2)
                .broadcast_to([128, nt, NS, NH])
            )
            nc.vector.tensor_mul(
                a4[:, 0:G].rearrange("p (t s h) -> p t s h", t=nt, s=NS),
                e_tsh, r4b
            )
            # p2 = attn * V, laid out (s, t, d, h) so the s-reduction tree is
            # contiguous halves; V columns are d-major so the attn broadcast
            # is stride-1 innermost (DVE 2x). One mul per t (3-dim AP limit).
            p2 = work.tile([128, 4096], BF, tag="p2", name=f"p2_{m}")
            p2v = p2[:, 0:nt * NS * 128].rearrange("p (s t c) -> p s t c",
                                                   s=NS, t=nt)
            a4r = a4[:, 0:G].rearrange("p (t s h) -> p t s h", t=nt, s=NS)
            vvr = kv4r[:, :, 128:KV].rearrange(
                "p t (s kv d h) -> p t s kv d h", s=NS, kv=2, d=HD
            )
            for t in range(nt):
                ab_t = (
                    a4r[:, t, :, :]
                    .unsqueeze(2)
                    .broadcast_to([128, NS, HD, NH])
                )
                nc.vector.tensor_mul(
                    p2v[:, :, t, :].rearrange("p s (d h) -> p s d h", d=HD),
                    ab_t, vvr[:, t, :, 1, :, :],
                )
            # s-reduction tree: big level on DVE (2x bf16); the small
            # contiguous 1-D tail levels on the idle GPSIMD
            cur, cn = p2, NS
            for r in range(3):
                nxt = work.tile(
                    [128, cn * 256], BF, tag=f"v{r}", name=f"v{r}_{m}"
                )
                W = cn * nt * 64
                nc.vector.tensor_add(
                    nxt[:, 0:W], cur[:, 0:W], cur[:, W:2 * W]
                )
                cur, cn = nxt, cn // 2
            st["wtd4"] = cur  # [128, nt*128] bf16, (t, c), c=(d,h) d-major

        def emit_transposes(m, st):
            # PE reaches these after the round's MMs; wtd4 (chainB, emitted
            # earlier this round on DVE) is ready by then. The wtdT4 SBUF
            # copy is DVE's last op of the round; its consumer (out-proj) is
            # scheduled at the TOP of the next PE round.
            nt = st["nt"]
            wtd4 = st["wtd4"]
            ptp4 = psO.tile([128, 512], BF, tag="ptp4", name=f"ptp4_{m}")
            for t in range(nt):
                nc.tensor.transpose(ptp4[:, t * 128:(t + 1) * 128],
                                    wtd4[:, t * 128:(t + 1) * 128],
                                    ident_sb[:])
            wtdT4 = work.tile([128, 512], BF, tag="wtdT4", name=f"wtdT4_{m}")
            nc.vector.tensor_copy(wtdT4[:, 0:nt * 128], ptp4[:, 0:nt * 128])
            st["wtdT4"] = wtdT4

        def emit_outproj(m, st):
            nt, row0 = st["nt"], st["row0"]
            wtdT4 = st["wtdT4"]
            po4 = psO.tile([128, 512], F32, tag="po4", name=f"po4_{m}")
            for t in range(nt):
                nc.tensor.matmul(po4[:, t * 128:(t + 1) * 128],
                                 lhsT=wtdT4[:, t * 128:(t + 1) * 128],
                                 rhs=wo_sb[:], start=True, stop=True)
            out_sb = osb.tile([128, 4 * TILE], F32, tag="out_sb",
                              name=f"osb{m}")
            nc.scalar.copy(out_sb[:, 0:nt * 128], po4[:, 0:nt * 128])
            if has_bias:
                nc.vector.tensor_add(
                    out_sb[:, 0:nt * 128].rearrange("p (t j) -> p t j", t=nt),
                    out_sb[:, 0:nt * 128].rearrange("p (t j) -> p t j", t=nt),
                    bo_sb[:, :].partition_broadcast(128).unsqueeze(1)
                    .broadcast_to([128, nt, 128]),
                )
            nc.sync.dma_start(
                outd[row0:row0 + nt * TILE, :].rearrange(
                    "(t p) j -> p t j", t=nt
                ),
                out_sb[:, 0:nt * 128].rearrange("p (t j) -> p t j", t=nt),
            )

        # Software pipeline at macro granularity. Emission order per round r:
        #   1. out-proj(r-2) + output copy/DMA   (inputs one round stale)
        #   2. loads + projection MMs + PSUM->SBUF copies (r)
        #   3. chainB(r-1): softmax tail + attn*V  (e4 one round stale)
        #   4. chainA(r): QK + d-reduce + exp     (kv4 copies same round)
        #   5. transposes(r-1)                    (wtd4 from step 3)
        # PE's stream [outproj | MMs | transposes] never waits on same-round
        # work; DVE's only same-round dependency (p1 on the copies) is
        # buffered by chainB work in front of it.
        # Macro schedule: short first/last macros (2 tiles) trim the
        # pipeline fill and drain; 512-row macros in the middle.
        if bs >= 2048 and (bs - 1024) % MACRO == 0:
            sizes = [256] + [512] * ((bs - 1024) // 512) + [256] * 3
        else:
            sizes = [512] * (bs // 512)
        macros = []
        row0 = 0
        for rows in sizes:
            macros.append((row0, rows))
            row0 += rows
        n_macro = len(macros)

        state = {}
        xts = {}

        def ensure_load(m):
            # Prefetch: issue macro m's load the round BEFORE its matmuls,
            # and ahead of the out-DMA trigger in the sync engine's in-order
            # stream (that trigger waits on same-round ScalarE work, which
            # used to delay the next load by most of a round).
            if m < n_macro and m not in xts:
                row0, rows = macros[m]
                xts[m] = emit_load(m, row0, rows)

        for r in range(n_macro + 2):
            ensure_load(r)
            ensure_load(r + 1)
            if 0 <= r - 2 < n_macro:
                emit_outproj(r - 2, state[r - 2])
                del state[r - 2]
            if r < n_macro:
                row0, rows = macros[r]
                nt = rows // TILE
                st = state[r] = {
                    "xt": xts.pop(r),
                    "kv4": kvp.tile([128, 4 * KV], BF, tag="kv4",
                                    name=f"kv4_{r}"),
                    "nt": nt,
                    "row0": row0,
                }
                for j in range(nt):
                    emit_mm_tile(r, j, st["xt"], st["kv4"], rows)
            if 0 <= r - 1 < n_macro:
                emit_chainB(r - 1, state[r - 1])
            if r < n_macro:
                emit_chainA(r, state[r])
            if 0 <= r - 1 < n_macro:
                emit_transposes(r - 1, state[r - 1])

    nc.compile()
    return nc


def _get_compiled(bs: int, has_bias: bool):
    key = (bs, has_bias)
    if key not in _compiled:
        _compiled[key] = _build(bs, has_bias)
    return _compiled[key]


def _pack_inputs(agent_obs, messages, Wq, bq, Wk, bk, Wv, bv, Wo, bo):
    """Host-side packing (per full batch): returns dict of device arrays."""
    bf16 = ml_dtypes.bfloat16
    b = agent_obs.shape[0]
    allm = np.concatenate([agent_obs[:, None, :], messages], axis=1)  # [b, 8, 256]
    # slab-chunk-major, feature-transposed: xpack[2s+ch, k, b]
    xpack = np.ascontiguousarray(
        allm.reshape(b, NS, 2, 128).transpose(1, 2, 3, 0).reshape(16, 128, b)
    ).astype(bf16)

    # V (and Wo rows) in d-major column order c' = d*NH + h so the DVE
    # attn broadcast is stride-1 innermost.
    perm = (np.arange(128).reshape(NH, HD).T).reshape(-1)  # c' -> h*HD+d
    WvTp = Wv.T[:, perm]
    wcat = np.concatenate([Wq.T, Wk.T, WvTp], axis=1)  # [256, 384]
    wpack = np.ascontiguousarray(
        wcat.reshape(2, 128, 384).transpose(1, 0, 2)
    ).astype(bf16)  # [128, 2, 384]
    wo = np.ascontiguousarray(Wo.T[perm, :]).astype(bf16)  # [128, 128]

    has_bias = bool(
        np.any(bq != 0) or np.any(bk != 0) or np.any(bv != 0) or np.any(bo != 0)
    )
    extra = {"ident": np.eye(128, dtype=bf16)}
    if has_bias:
        # PSUM layout [Q | K0 V0 | ... | K7 V7]
        bkv = np.zeros((1, 2176), np.float32)
        bkv[0, 0:128] = bq
        for s in range(NS):
            bkv[0, 128 + s * 256:128 + s * 256 + 128] = bk
            bkv[0, 256 + s * 256:256 + s * 256 + 128] = bv[perm]
        extra["bkv"] = bkv
        extra["bo"] = bo.reshape(1, 128).astype(np.float32)
    return xpack, wpack, wo, extra, has_bias


def kernel(agent_obs, messages, Wq, bq, Wk, bk, Wv, bv, Wo, bo):
    b = agent_obs.shape[0]
    assert b % N_CORES == 0
    bs = b // N_CORES

    xpack, wpack, wo, extra, has_bias = _pack_inputs(
        np.asarray(agent_obs, np.float32), np.asarray(messages, np.float32),
        np.asarray(Wq, np.float32), np.asarray(bq, np.float32),
        np.asarray(Wk, np.float32), np.asarray(bk, np.float32),
        np.asarray(Wv, np.float32), np.asarray(bv, np.float32),
        np.asarray(Wo, np.float32), np.asarray(bo, np.float32),
    )
    nc = _get_compiled(bs, has_bias)

    in_maps = []
    for c in range(N_CORES):
        m = {
            "xpack": np.ascontiguousarray(xpack[:, :, c * bs:(c + 1) * bs]),
            "wpack": wpack,
            "wo": wo,
        }
        m.update(extra)
        in_maps.append(m)

    res = run_bass_kernel_spmd(nc, in_maps, core_ids=list(range(N_CORES)))
    out = np.concatenate([r["out"] for r in res.results], axis=0)
    return out.astype(np.float32)

